# revision 32
# baseline (speedup 1.0000x reference)
"""Trainium2 Bass kernel for KernelAttention (gaussian-kernel multi-head attention).

Math (per batch b):
  d2[q,k]   = |q_pos[q] - k_pos[k]|^2   (as -d2 via one K=15 hi/lo bf16 matmul)
  s_h[k,q]  = exp(-c_h * d2),  c_h = 1/lengthscale_h^2
  att_h[q,v]= sum_k s_h[k,q] * V[k,h,v] / (sum_k s_h[k,q] + 1e-5)
  out[o,q]  = sum_{h,v} w_out[o, h*64+v] * att_h[q,v]

Key optimizations over a direct implementation:
  * Mask compaction on host: only unmasked keys (~1024 of 2048) are shipped,
    so score volume, exp work and attend matmuls all halve (KT 16 -> 9).
  * Only 3 ACT exps (c=25, 4, 0.25); c=100 and c=1 are derived by bf16 DVE
    squarings (s^4 = (s^2)^2), emitted per 3-ktile group so they hide
    behind the ACT exps of later k-tiles.
  * Diffuse heads (c <= 0.05) use a low-rank polynomial factorization:
    exp(-c d2) = phi(q).psi(k) with damped-monomial features (deg 5/4/3,
    111 shared feature rows), replacing 3 full score matrices with tiny
    matmuls.  Taylor truncation error < 1e-3 on the attended values.
  * d2 is consumed by ACT directly from PSUM (no PSUM->SBUF evacuation);
    d2 tiles are double-buffered so ACT (the phase-A pacer) never stalls.
  * Two attends accumulate in-phase; the rest pipeline through both PSUM
    pools right after, ordered so late heads do not gate early norms.
  * Normalization deferred past attend via a ones-column (psum row 64);
    eps is folded into the evacuation copy as a per-partition bias;
    r = 1/(norm+eps) via the table-free DVE reciprocal approximation
    (avoids ACT Ln/Exp activation-table swaps, 1.3us each); r broadcast
    across partitions with a tiny K=4 matmul, split in two row groups so
    early flat tiles are scaled before the last head finishes.
  * Input/output DMAs spread across the three DMA-capable queues.

Sharding: 8 cores = (batch b in 0..3) x (query half in 0..1); each core owns
[1024 q, ~1152 compacted k].  No collectives; outputs gathered on host.
"""

import numpy as np
from contextlib import ExitStack
from math import factorial

B, LQ, LK, DPOS = 4, 2048, 2048, 3
H, V, OUTD = 8, 64, 512
QS = LQ // 2          # q rows per core
V1 = V + 1            # value cols + ones col
NCORES = 8

_cache = {}


def _chain_plan(cv):
    """Returns (poly_heads, score_heads, exp_heads, derived) given coeffs.

    poly_heads: heads with c small enough for degree<=5 Taylor factorization.
    derived: head -> source head with c_head = 4*c_source (s_head = s_src^4).
    """
    poly = {}
    for h, c in enumerate(cv):
        # degrees validated numerically for randn(3) positions (|q.k| <~ 20):
        # attended error <= 3e-4 for c in {0.04, 0.01, 0.0025}
        if c <= 0.05:
            poly[h] = 5 if c > 0.02 else (4 if c > 0.005 else 3)
    # feature rows must fit in 128 partitions; drop the widest poly heads
    # back to the explicit-score path if an unusual lengthscale set overflows
    def _ft():
        return sum(len(_monomials(d)) for d in poly.values())
    while poly and _ft() > 128:
        del poly[max(poly, key=lambda h: poly[h])]
    score = [h for h in range(len(cv)) if h not in poly]
    # depth-1 chains only: a head may be derived (s = src^4) only from a
    # head that is itself exp'd, so bf16 squaring error stays ~1%.
    derived = {}
    exp_heads = []
    for h in sorted(score, key=lambda h: cv[h]):   # increasing sharpness
        src = next((s for s in exp_heads
                    if np.float32(cv[h]) == np.float32(4.0) * np.float32(cv[s])),
                   None)
        if src is not None:
            derived[h] = src
        else:
            exp_heads.append(h)
    return poly, score, exp_heads, derived


def _order_score_heads(exp_heads, derived):
    """Process exp'd heads first, then derived in dependency order."""
    order = list(exp_heads)
    rest = dict(derived)
    while rest:
        for h, src in list(rest.items()):
            if src in order:
                order.append(h)
                del rest[h]
    return order


def _monomials(deg):
    out = []
    for a in range(deg + 1):
        for b in range(deg + 1 - a):
            for c in range(deg + 1 - a - b):
                out.append((a, b, c))
    return out


def _features(pos, c, deg):
    """Damped-monomial features: f_a(x) = sqrt((2c)^j/(a!b!c!)) x^a exp(-c|x|^2)."""
    mons = _monomials(deg)
    p = pos.astype(np.float64)
    damp = np.exp(-np.float64(c) * (p ** 2).sum(-1))
    cols = []
    for (a, b, cc) in mons:
        j = a + b + cc
        coef = np.sqrt((2 * np.float64(c)) ** j /
                       (factorial(a) * factorial(b) * factorial(cc)))
        cols.append(coef * p[:, 0] ** a * p[:, 1] ** b * p[:, 2] ** cc * damp)
    return np.stack(cols, -1).astype(np.float32)  # [N, F]


def _build(key_cv, KT, poly, score, exp_heads, derived, fdims):
    key = (key_cv, KT)
    if key in _cache:
        return _cache[key]
    import concourse.bacc as bacc
    import concourse.tile as tile
    from concourse import mybir

    f32 = mybir.dt.float32
    f32r = mybir.dt.float32r
    bf16 = mybir.dt.bfloat16
    AF = mybir.ActivationFunctionType
    cv = list(key_cv)

    NS = len(score)            # score (explicit) heads
    NP = len(poly)             # poly heads
    FT = sum(fdims[h] for h in poly)   # total feature rows (<=128)
    PV = NP * V1               # poly aug-value cols
    LKp = KT * 128
    order = _order_score_heads(exp_heads, derived)
    scol = {h: i for i, h in enumerate(order)}   # vp column block per head

    nc = bacc.Bacc("TRN2", target_bir_lowering=False, debug=False,
                   num_devices=NCORES)
    # hi/lo bf16 split of the K=5 augmented distance operands:
    # rows [hi(5); lo(5); hi(5)] x [hi(5); hi(5); lo(5)] accumulate
    # hi*hi + lo*hi + hi*lo in f32 PSUM (lo*lo dropped).
    ka = nc.dram_tensor("ka", [15, LKp], bf16, kind="ExternalInput").ap()
    qa = nc.dram_tensor("qa", [15, QS], bf16, kind="ExternalInput").ap()
    vp = nc.dram_tensor("vp", [128, KT, NS * V1], bf16, kind="ExternalInput").ap()
    if poly:
        vaug = nc.dram_tensor("vaug", [128, KT, PV], bf16,
                              kind="ExternalInput").ap()
        psi = nc.dram_tensor("psi", [128, KT, FT], bf16,
                             kind="ExternalInput").ap()
    phis = {h: nc.dram_tensor(f"phi{h}", [fdims[h], QS], bf16,
                              kind="ExternalInput").ap() for h in poly}
    wt = nc.dram_tensor("wt", [128, 4, OUTD], bf16, kind="ExternalInput").ap()
    sel4 = nc.dram_tensor("sel4", [4, 4, 128], bf16, kind="ExternalInput").ap()
    outT = nc.dram_tensor("outT", [OUTD, QS], bf16, kind="ExternalOutput").ap()

    with tile.TileContext(nc) as tc, ExitStack() as ctx:
        const = ctx.enter_context(tc.tile_pool(name="const", bufs=1))
        spool = ctx.enter_context(tc.tile_pool(name="spool", bufs=1))
        tmp = ctx.enter_context(tc.tile_pool(name="tmp", bufs=2))
        fpool = ctx.enter_context(tc.tile_pool(name="fpool", bufs=4))
        obuf = ctx.enter_context(tc.tile_pool(name="obuf", bufs=2))
        # PSUM: psA 2x[128,1024]f32 (4 banks) + psB 3x[65|128,1024] -> but
        # only 2 psB slots fit beside a double-buffered psA... we keep psA
        # bufs=2 (4 banks) and psB bufs=2 (4 banks) during phase A by
        # dedicating the psB slots to the two in-phase attends; the third
        # attend (first derived head) accumulates per-group into psA's
        # second slot... no: psA slots rotate with d2.  Final split:
        # psA bufs=2 (d2 double-buffer, 4 banks), psB bufs=2 (4 banks).
        psA = ctx.enter_context(tc.tile_pool(name="psA", bufs=2, space="PSUM"))
        psB = ctx.enter_context(tc.tile_pool(name="psB", bufs=2, space="PSUM"))

        # input DMAs spread over queues; ka/qa first (gate the dist matmul)
        ka_sb = const.tile([15, LKp], bf16)
        nc.sync.dma_start(out=ka_sb[:, 0:128], in_=ka[:, 0:128])
        qa_sb = const.tile([15, QS], bf16)
        nc.sync.dma_start(out=qa_sb[:], in_=qa)
        nc.sync.dma_start(out=ka_sb[:, 128:LKp], in_=ka[:, 128:LKp])
        vp_sb = const.tile([128, KT, NS * V1], bf16)
        nc.scalar.dma_start(out=vp_sb[:], in_=vp)
        wt_sb = const.tile([128, 4, OUTD], bf16)
        nc.sync.dma_start(out=wt_sb[:], in_=wt)
        sel4_sb = const.tile([4, 4, 128], bf16)
        nc.scalar.dma_start(out=sel4_sb[:], in_=sel4)
        if poly:
            vaug_sb = const.tile([128, KT, PV], bf16)
            nc.gpsimd.dma_start(out=vaug_sb[:], in_=vaug)
            psi_sb = const.tile([128, KT, FT], bf16)
            nc.gpsimd.dma_start(out=psi_sb[:], in_=psi)
        phi_sb = {}
        for h in poly:
            phi_sb[h] = const.tile([fdims[h], QS], bf16, name=f"phi{h}")
            nc.gpsimd.dma_start(out=phi_sb[h][:], in_=phis[h])

        # norms/r split into two 4-row groups (heads 4-7 / heads 0-3) so
        # each group's Ln/Exp can run as soon as its heads evacuate; engine
        # partition slices must start at partition 0, hence separate tiles.
        normsA = const.tile([4, QS], f32)
        normsB = const.tile([4, QS], f32)
        # eps folded into the evac copy: bias column is 1e-5 only on the
        # normalizer row, so norm arrives as (sum_k s) + 1e-5
        eps_col = const.tile([V1, 1], f32)
        nc.vector.memset(eps_col[:], 0.0)
        nc.vector.memset(eps_col[64:65, :], 1e-5)
        rfA = const.tile([4, QS], f32)
        rfB = const.tile([4, QS], f32)
        r_hiA = const.tile([4, QS], bf16)
        r_hiB = const.tile([4, QS], bf16)
        flat = [const.tile([128, QS], bf16, name=f"flat{j}") for j in range(4)]

        s_tiles = {h: spool.tile([128, KT, QS], bf16, name=f"s{h}")
                   for h in score}

        def att_mms(att, h, kt, start, stop):
            c0 = scol[h] * V1
            for qc in range(2):
                s5 = slice(qc * 512, (qc + 1) * 512)
                nc.tensor.matmul(att[:, s5],
                                 lhsT=vp_sb[:, kt, c0:c0 + V1],
                                 rhs=s_tiles[h][:, kt, s5],
                                 start=start, stop=stop)

        def evac(h, att, act):
            # one copy [65, QS]: rows 0..63 attended, row 64 normalizer
            fh = fpool.tile([V1, QS], bf16, tag="fh", name=f"fh{h}")
            if act:
                nc.scalar.activation(out=fh[:], in_=att[:], func=AF.Identity,
                                     bias=eps_col[:])
            else:
                nc.vector.tensor_scalar_add(out=fh[:], in0=att[:],
                                            scalar1=eps_col[:])
            r0 = (h % 2) * 64
            nc.sync.dma_start(out=flat[h // 2][r0:r0 + 64, :], in_=fh[0:64, :])
            # casting DMA (bf16 -> f32) must go through gpsimd
            nt = normsA if h >= 4 else normsB
            nc.gpsimd.dma_start(out=nt[h % 4:h % 4 + 1, :], in_=fh[64:65, :])

        # ---- phase A: dist matmul -> 3 exps per k-tile; d2 double-buffered
        # so ACT never waits on the PE.  The first two exp'd heads attend
        # in-phase (2 psB slots); chain squarings for the derived heads run
        # on DVE per 3-ktile group, hidden behind ACT.  The remaining exp'd
        # head and the derived heads attend after the loop.
        inphase = exp_heads[:2]
        postexp = exp_heads[2:]
        atts = {h: psB.tile([V1, QS], f32, tag="att", name=f"att{h}")
                for h in inphase}
        GK = 3
        dorder = [h for h in order if h in derived]
        for kt in range(KT):
            d2 = psA.tile([128, QS], f32, tag="ps")
            for qc in range(2):
                s5 = slice(qc * 512, (qc + 1) * 512)
                nc.tensor.matmul(d2[:, s5],
                                 lhsT=ka_sb[:, kt * 128:(kt + 1) * 128],
                                 rhs=qa_sb[:, s5], start=True, stop=True)
            for h in exp_heads:
                nc.scalar.activation(out=s_tiles[h][:, kt, :], in_=d2[:],
                                     func=AF.Exp, scale=float(cv[h]))
            for h in inphase:
                att_mms(atts[h], h, kt, start=(kt == 0), stop=(kt == KT - 1))
            if kt % GK == GK - 1 or kt == KT - 1:
                g0 = kt - kt % GK
                gs = slice(g0, kt + 1)
                gn = kt + 1 - g0
                for h in dorder:
                    src = s_tiles[derived[h]]
                    t = tmp.tile([128, gn, QS], bf16, tag="tmp",
                                 name=f"t{h}_{g0}")
                    nc.vector.tensor_mul(t[:], src[:, gs, :], src[:, gs, :])
                    nc.vector.tensor_mul(s_tiles[h][:, gs, :], t[:], t[:])
        for i, h in enumerate(inphase):
            evac(h, atts[h], act=(i == 0))

        # ---- poly heads: W[f, v] = sum_k psi[k, f] vaug[k, v]  (fills the
        # PE gap right after phase A)
        Wh = {}
        if poly:
            Wp = psA.tile([FT, PV], f32, tag="ps", name="Wp")
            for kt in range(KT):
                nc.tensor.matmul(Wp[:], lhsT=psi_sb[:, kt, :],
                                 rhs=vaug_sb[:, kt, :],
                                 start=(kt == 0), stop=(kt == KT - 1))
            W_sb = const.tile([FT, PV], bf16)
            nc.vector.tensor_copy(out=W_sb[:], in_=Wp[:])
            # per-head W slices shifted to partition 0 (DMA moves partitions)
            r0 = 0
            for i, h in enumerate(sorted(poly)):
                F = fdims[h]
                Wh[h] = const.tile([F, V1], bf16, name=f"W{h}")
                nc.sync.dma_start(out=Wh[h][:],
                                  in_=W_sb[r0:r0 + F, i * V1:(i + 1) * V1])
                r0 += F

        # ---- remaining attends.  The exp'd leftover + derived heads go
        # through psA (idle after phase A) so up to 4 attends are in
        # flight; poly attends rotate through psB ahead of the rb tiles.
        # poly attends first: they are cheap (2 matmuls each) and their
        # norms gate the first reciprocal group, so get them out of the way
        for h in sorted(poly):
            att = psB.tile([V1, QS], f32, tag="att", name=f"att{h}")
            for qc in range(2):
                s5 = slice(qc * 512, (qc + 1) * 512)
                nc.tensor.matmul(att[:, s5], lhsT=Wh[h][:],
                                 rhs=phi_sb[h][:, s5],
                                 start=True, stop=True)
            evac(h, att, act=(h == max(poly)))
        for i, h in enumerate(postexp + dorder):
            att = psA.tile([V1, QS], f32, tag="ps", name=f"att{h}")
            for kt in range(KT):
                att_mms(att, h, kt, start=(kt == 0), stop=(kt == KT - 1))
            evac(h, att, act=(i != 1))

        # ---- normalization: r = 1/(norm + 1e-5) via exp(-ln(x)), done in
        # two row-group passes so early heads don't wait for late ones.
        # r is broadcast across partitions by stride-0 DMA (no PSUM/PE),
        # then flat is scaled on DVE (bf16 2x).
        flatn = [const.tile([128, QS], bf16, name=f"flatn{j}")
                 for j in range(4)]
        for p, (nt, rf, rt) in enumerate(((normsA, rfA, r_hiA),
                                          (normsB, rfB, r_hiB))):
            # r = 1/(norm + eps): table-free DVE approx (~18 bits) instead of
            # ACT Ln/Exp, which would swap activation tables (1.3us each)
            nc.vector.reciprocal_approx_fast(out=rf[:], in_=nt[:])
            nc.vector.tensor_copy(out=rt[:], in_=rf[:])
            for j in ((2, 3) if p == 0 else (0, 1)):
                rb = (psB if p == 0 else psA).tile(
                    [128, QS], f32, tag="att" if p == 0 else "ps",
                    name=f"rb{j}")
                for qc in range(2):
                    s5 = slice(qc * 512, (qc + 1) * 512)
                    nc.tensor.matmul(rb[:, s5], lhsT=sel4_sb[:, j, :],
                                     rhs=rt[:, s5], start=True, stop=True)
                nc.vector.tensor_mul(flatn[j][:], flat[j][:], rb[:])

        # ---- out projection: outT[o, q] = sum_hv wt[hv, o] * flatn[hv, q]
        oq = [nc.sync, nc.scalar, nc.gpsimd, nc.sync]
        pools = [psA, psA, psB, psB]
        for ot in range(4):
            po = pools[ot].tile([128, QS], f32,
                                tag="ps" if ot < 2 else "att", name=f"po{ot}")
            for j in [2, 3, 0, 1]:
                for qc in range(2):
                    s5 = slice(qc * 512, (qc + 1) * 512)
                    nc.tensor.matmul(po[:, s5],
                                     lhsT=wt_sb[:, j, ot * 128:(ot + 1) * 128],
                                     rhs=flatn[j][:, s5],
                                     start=(j == 2), stop=(j == 1))
            ob = obuf.tile([128, QS], bf16, tag="ob", name=f"ob{ot}")
            if ot % 2 == 0:
                nc.scalar.copy(out=ob[:], in_=po[:])
            else:
                nc.vector.tensor_copy(out=ob[:], in_=po[:])
            oq[ot].dma_start(out=outT[ot * 128:(ot + 1) * 128, :], in_=ob[:])

    nc.compile()
    _cache[key] = nc
    return nc


def _hilo(x, bf16):
    hi = x.astype(bf16)
    lo = (x - hi.astype(np.float32)).astype(bf16)
    return hi, lo


def _prep_batch(kpos, vv, KT, cvf, poly, order, fdims, bf16):
    """Per-batch (key-side) tensors: ka, vp, vaug, psi."""
    Kp = KT * 128
    ncnt = kpos.shape[0]
    NS = len(order)
    k2 = (kpos * kpos).sum(-1)
    ka5 = np.zeros((5, Kp), np.float32)
    ka5[0:3, :ncnt] = kpos.T
    ka5[3, :ncnt] = k2
    ka5[4, :ncnt] = 1.0
    ka_hi, ka_lo = _hilo(ka5, bf16)
    ka = np.concatenate([ka_hi, ka_lo, ka_hi])   # [15, Kp]

    # score-head values (+ones), padded, [128, KT, NS*V1] fp16
    vs = np.zeros((Kp, NS, V1), np.float32)
    for i, h in enumerate(order):
        vs[:ncnt, i, :V] = vv[:, h, :]
    vs[:ncnt, :, V] = 1.0
    vp = vs.reshape(KT, 128, NS * V1).transpose(1, 0, 2).astype(bf16)

    # poly-head aug values + features
    ph = sorted(poly)
    va = np.zeros((Kp, len(ph), V1), np.float32)
    for i, h in enumerate(ph):
        va[:ncnt, i, :V] = vv[:, h, :]
    va[:ncnt, :, V] = 1.0
    vaug = va.reshape(KT, 128, len(ph) * V1).transpose(1, 0, 2).astype(bf16)
    FT = sum(fdims[h] for h in ph)
    psi = np.zeros((Kp, FT), np.float32)
    c0 = 0
    for h in ph:
        psi[:ncnt, c0:c0 + fdims[h]] = _features(kpos, cvf[h], poly[h])
        c0 += fdims[h]
    psi = psi.reshape(KT, 128, FT).transpose(1, 0, 2).astype(bf16)
    out = {"ka": np.ascontiguousarray(ka), "vp": np.ascontiguousarray(vp)}
    if ph:
        out["vaug"] = np.ascontiguousarray(vaug)
        out["psi"] = np.ascontiguousarray(psi)
    return out


def _prep_core(qp, cvf, poly, fdims, bf16):
    """Per-core (query-side) tensors: qa, phi{h}."""
    q2 = (qp * qp).sum(-1)
    one_q = np.ones(QS, np.float32)
    qa5 = np.stack([2 * qp[:, 0], 2 * qp[:, 1], 2 * qp[:, 2], -one_q, -q2]) \
        .astype(np.float32)
    qa_hi, qa_lo = _hilo(qa5, bf16)
    qa = np.concatenate([qa_hi, qa_hi, qa_lo])   # [15, QS]
    out = {"qa": np.ascontiguousarray(qa)}
    for h in sorted(poly):
        out[f"phi{h}"] = np.ascontiguousarray(
            _features(qp, cvf[h], poly[h]).T.astype(bf16))
    return out


def kernel(query_positions, key_positions, values, masked_elements,
           lengthscales, w_out, _want_trace=False):
    import ml_dtypes
    from concourse.bass_utils import run_bass_kernel_spmd

    bf16 = ml_dtypes.bfloat16
    qp = np.asarray(query_positions, np.float32)
    kp = np.asarray(key_positions, np.float32)
    vals = np.asarray(values, np.float32)
    mask = np.asarray(masked_elements).astype(bool)
    ls = np.asarray(lengthscales, np.float32)
    w = np.asarray(w_out, np.float32)

    cvf = (1.0 / (ls.astype(np.float64) ** 2)).astype(np.float32)
    poly, score, exp_heads, derived = _chain_plan(cvf)
    order = _order_score_heads(exp_heads, derived)
    fdims = {h: len(_monomials(d)) for h, d in poly.items()}

    keeps = [np.where(~mask[b])[0] for b in range(B)]
    KT = max(1, int(np.ceil(max(len(k) for k in keeps) / 128)))

    nc = _build(tuple(float(x) for x in cvf), KT, poly, score, exp_heads,
                derived, fdims)

    # shared (head-side) tensors
    wt = np.ascontiguousarray(w.T).reshape(4, 128, OUTD) \
        .transpose(1, 0, 2).astype(bf16)
    # sel4[:, j, :] broadcasts the two r rows of flat-group j (rows are
    # head%4 within the high/low norm group) across 128 partitions
    sel4 = np.zeros((4, 4, 128), np.float32)
    for j in range(4):
        h0, h1 = 2 * j, 2 * j + 1
        sel4[h0 % 4, j, :64] = 1.0
        sel4[h1 % 4, j, 64:] = 1.0
    shared = {"wt": np.ascontiguousarray(wt), "sel4": sel4.astype(bf16)}

    batch_maps = []
    for b in range(B):
        batch_maps.append(_prep_batch(kp[b][keeps[b]], vals[b][keeps[b]],
                                      KT, cvf, poly, order, fdims, bf16))
    in_maps = []
    for c in range(NCORES):
        b, hf = c // 2, c % 2
        m = dict(shared)
        m.update(batch_maps[b])
        m.update(_prep_core(qp[b, hf * QS:(hf + 1) * QS], cvf, poly, fdims,
                            bf16))
        in_maps.append(m)
    res = run_bass_kernel_spmd(nc, in_maps, core_ids=list(range(NCORES)),
                               trace=_want_trace)
    out = np.empty((B, LQ, OUTD), np.float32)
    for c in range(NCORES):
        b, hf = c // 2, c % 2
        out[b, hf * QS:(hf + 1) * QS, :] = \
            res.results[c]["outT"].astype(np.float32).T
    if _want_trace:
        return out, res
    return out



# revision 34
# speedup vs baseline: 1.1909x; 1.1909x over previous
"""Trainium2 Bass kernel for KernelAttention (gaussian-kernel multi-head attention).

Math (per batch b):
  d2[q,k]   = |q_pos[q] - k_pos[k]|^2   (as -d2 via one K=15 hi/lo bf16 matmul)
  s_h[k,q]  = exp(-c_h * d2),  c_h = 1/lengthscale_h^2
  att_h[q,v]= sum_k s_h[k,q] * V[k,h,v] / (sum_k s_h[k,q] + 1e-5)
  out[o,q]  = sum_{h,v} w_out[o, h*64+v] * att_h[q,v]

Key optimizations over a direct implementation:
  * Mask compaction on host: only unmasked keys (~1024 of 2048) are shipped,
    so score volume, exp work and attend matmuls all halve (KT 16 -> 9).
  * Only 3 ACT exps (c=25, 4, 0.25); c=100 and c=1 are derived by bf16 DVE
    squarings (s^4 = (s^2)^2), emitted per 3-ktile group so they hide
    behind the ACT exps of later k-tiles.
  * Diffuse heads (c <= 0.05) use a low-rank polynomial factorization:
    exp(-c d2) = phi(q).psi(k) with damped-monomial features (deg 5/4/3,
    111 shared feature rows), replacing 3 full score matrices with tiny
    matmuls.  Taylor truncation error < 1e-3 on the attended values.
  * d2 is consumed by ACT directly from PSUM (no PSUM->SBUF evacuation);
    d2 tiles are double-buffered so ACT (the phase-A pacer) never stalls.
  * Two attends accumulate in-phase; the rest pipeline through both PSUM
    pools right after, ordered so late heads do not gate early norms.
  * Normalization deferred past attend via a ones-column (psum row 64);
    eps is folded into the evacuation copy as a per-partition bias;
    r = 1/(norm+eps) via the table-free DVE reciprocal approximation
    (avoids ACT Ln/Exp activation-table swaps, 1.3us each); r broadcast
    across partitions with a tiny K=4 matmul, split in two row groups so
    early flat tiles are scaled before the last head finishes.
  * Input/output DMAs spread across the three DMA-capable queues.

Sharding: 8 cores = (batch b in 0..3) x (query half in 0..1); each core owns
[1024 q, ~1152 compacted k].  No collectives; outputs gathered on host.
"""

import numpy as np
from contextlib import ExitStack
from math import factorial

B, LQ, LK, DPOS = 4, 2048, 2048, 3
H, V, OUTD = 8, 64, 512
QS = LQ // 2          # q rows per core
V1 = V + 1            # value cols + ones col
NCORES = 8

_cache = {}


def _chain_plan(cv):
    """Returns (poly_heads, score_heads, exp_heads, derived) given coeffs.

    poly_heads: heads with c small enough for degree<=5 Taylor factorization.
    derived: head -> source head with c_head = 4*c_source (s_head = s_src^4).
    """
    poly = {}
    for h, c in enumerate(cv):
        # degrees validated numerically for randn(3) positions (|q.k| <~ 20):
        # attended error <= 3e-4 for c in {0.04, 0.01, 0.0025}
        if c <= 0.05:
            poly[h] = 5 if c > 0.02 else (4 if c > 0.005 else 3)
    # feature rows must fit in 128 partitions; drop the widest poly heads
    # back to the explicit-score path if an unusual lengthscale set overflows
    def _ft():
        return sum(len(_monomials(d)) for d in poly.values())
    while poly and _ft() > 128:
        del poly[max(poly, key=lambda h: poly[h])]
    score = [h for h in range(len(cv)) if h not in poly]
    # depth-1 chains only: a head may be derived (s = src^4) only from a
    # head that is itself exp'd, so bf16 squaring error stays ~1%.
    derived = {}
    exp_heads = []
    for h in sorted(score, key=lambda h: cv[h]):   # increasing sharpness
        src = next((s for s in exp_heads
                    if np.float32(cv[h]) == np.float32(4.0) * np.float32(cv[s])),
                   None)
        if src is not None:
            derived[h] = src
        else:
            exp_heads.append(h)
    return poly, score, exp_heads, derived


def _order_score_heads(exp_heads, derived):
    """Process exp'd heads first, then derived in dependency order."""
    order = list(exp_heads)
    rest = dict(derived)
    while rest:
        for h, src in list(rest.items()):
            if src in order:
                order.append(h)
                del rest[h]
    return order


def _monomials(deg):
    out = []
    for a in range(deg + 1):
        for b in range(deg + 1 - a):
            for c in range(deg + 1 - a - b):
                out.append((a, b, c))
    return out


def _features(pos, c, deg):
    """Damped-monomial features: f_a(x) = sqrt((2c)^j/(a!b!c!)) x^a exp(-c|x|^2)."""
    mons = _monomials(deg)
    p = pos.astype(np.float64)
    damp = np.exp(-np.float64(c) * (p ** 2).sum(-1))
    cols = []
    for (a, b, cc) in mons:
        j = a + b + cc
        coef = np.sqrt((2 * np.float64(c)) ** j /
                       (factorial(a) * factorial(b) * factorial(cc)))
        cols.append(coef * p[:, 0] ** a * p[:, 1] ** b * p[:, 2] ** cc * damp)
    return np.stack(cols, -1).astype(np.float32)  # [N, F]


def _build(key_cv, KT, poly, score, exp_heads, derived, fdims):
    key = (key_cv, KT)
    if key in _cache:
        return _cache[key]
    import concourse.bacc as bacc
    import concourse.tile as tile
    from concourse import mybir

    f32 = mybir.dt.float32
    f32r = mybir.dt.float32r
    bf16 = mybir.dt.bfloat16
    AF = mybir.ActivationFunctionType
    cv = list(key_cv)

    NS = len(score)            # score (explicit) heads
    NP = len(poly)             # poly heads
    FT = sum(fdims[h] for h in poly)   # total feature rows (<=128)
    PV = NP * V1               # poly aug-value cols
    LKp = KT * 128
    order = _order_score_heads(exp_heads, derived)
    scol = {h: i for i, h in enumerate(order)}   # vp column block per head

    nc = bacc.Bacc("TRN2", target_bir_lowering=False, debug=False,
                   num_devices=NCORES)
    # hi/lo bf16 split of the K=5 augmented distance operands:
    # rows [hi(5); lo(5); hi(5)] x [hi(5); hi(5); lo(5)] accumulate
    # hi*hi + lo*hi + hi*lo in f32 PSUM (lo*lo dropped).
    ka = nc.dram_tensor("ka", [15, LKp], bf16, kind="ExternalInput").ap()
    qa = nc.dram_tensor("qa", [15, QS], bf16, kind="ExternalInput").ap()
    vp = nc.dram_tensor("vp", [128, KT, NS * V1], bf16, kind="ExternalInput").ap()
    if poly:
        vaug = nc.dram_tensor("vaug", [128, KT, PV], bf16,
                              kind="ExternalInput").ap()
        psi = nc.dram_tensor("psi", [128, KT, FT], bf16,
                             kind="ExternalInput").ap()
    phis = {h: nc.dram_tensor(f"phi{h}", [fdims[h], QS], bf16,
                              kind="ExternalInput").ap() for h in poly}
    wt = nc.dram_tensor("wt", [128, 4, OUTD], bf16, kind="ExternalInput").ap()
    sel4 = nc.dram_tensor("sel4", [4, 4, 128], bf16, kind="ExternalInput").ap()
    outT = nc.dram_tensor("outT", [OUTD, QS], bf16, kind="ExternalOutput").ap()

    with tile.TileContext(nc) as tc, ExitStack() as ctx:
        const = ctx.enter_context(tc.tile_pool(name="const", bufs=1))
        spool = ctx.enter_context(tc.tile_pool(name="spool", bufs=1))
        tmp = ctx.enter_context(tc.tile_pool(name="tmp", bufs=2))
        fpool = ctx.enter_context(tc.tile_pool(name="fpool", bufs=4))
        obuf = ctx.enter_context(tc.tile_pool(name="obuf", bufs=2))
        # PSUM: psA 2x[128,1024]f32 (4 banks) + psB 3x[65|128,1024] -> but
        # only 2 psB slots fit beside a double-buffered psA... we keep psA
        # bufs=2 (4 banks) and psB bufs=2 (4 banks) during phase A by
        # dedicating the psB slots to the two in-phase attends; the third
        # attend (first derived head) accumulates per-group into psA's
        # second slot... no: psA slots rotate with d2.  Final split:
        # psA bufs=2 (d2 double-buffer, 4 banks), psB bufs=2 (4 banks).
        psA = ctx.enter_context(tc.tile_pool(name="psA", bufs=2, space="PSUM"))
        psB = ctx.enter_context(tc.tile_pool(name="psB", bufs=2, space="PSUM"))

        # input DMAs spread over queues; ka/qa first (gate the dist matmul)
        warm_src = const.tile([128, 144], bf16)
        nc.vector.memset(warm_src[:], 0.0)
        wps = psA.tile([16, 128], f32, tag="ps", name="warm")
        for _ in range(46):
            nc.tensor.matmul(wps[:], lhsT=warm_src[:, 0:16],
                             rhs=warm_src[:, 16:144], start=True, stop=True,
                             skip_group_check=True)
        ka_sb = const.tile([15, LKp], bf16)
        nc.sync.dma_start(out=ka_sb[:, 0:128], in_=ka[:, 0:128])
        qa_sb = const.tile([15, QS], bf16)
        nc.sync.dma_start(out=qa_sb[:], in_=qa)
        nc.sync.dma_start(out=ka_sb[:, 128:LKp], in_=ka[:, 128:LKp])
        vp_sb = const.tile([128, KT, NS * V1], bf16)
        nc.scalar.dma_start(out=vp_sb[:], in_=vp)
        wt_sb = const.tile([128, 4, OUTD], bf16)
        nc.sync.dma_start(out=wt_sb[:], in_=wt)
        sel4_sb = const.tile([4, 4, 128], bf16)
        nc.scalar.dma_start(out=sel4_sb[:], in_=sel4)
        if poly:
            vaug_sb = const.tile([128, KT, PV], bf16)
            nc.gpsimd.dma_start(out=vaug_sb[:], in_=vaug)
            psi_sb = const.tile([128, KT, FT], bf16)
            nc.gpsimd.dma_start(out=psi_sb[:], in_=psi)
        phi_sb = {}
        for h in poly:
            phi_sb[h] = const.tile([fdims[h], QS], bf16, name=f"phi{h}")
            nc.gpsimd.dma_start(out=phi_sb[h][:], in_=phis[h])

        # norms/r split into two 4-row groups (heads 4-7 / heads 0-3) so
        # each group's Ln/Exp can run as soon as its heads evacuate; engine
        # partition slices must start at partition 0, hence separate tiles.
        normsA = const.tile([4, QS], f32)
        normsB = const.tile([4, QS], f32)
        # eps folded into the evac copy: bias column is 1e-5 only on the
        # normalizer row, so norm arrives as (sum_k s) + 1e-5
        eps_col = const.tile([V1, 1], f32)
        nc.vector.memset(eps_col[:], 0.0)
        nc.vector.memset(eps_col[64:65, :], 1e-5)
        rfA = const.tile([4, QS], f32)
        rfB = const.tile([4, QS], f32)
        r_hiA = const.tile([4, QS], bf16)
        r_hiB = const.tile([4, QS], bf16)
        flat = [const.tile([128, QS], bf16, name=f"flat{j}") for j in range(4)]

        s_tiles = {h: spool.tile([128, KT, QS], bf16, name=f"s{h}")
                   for h in score}

        def att_mms(att, h, kt, start, stop):
            c0 = scol[h] * V1
            for qc in range(2):
                s5 = slice(qc * 512, (qc + 1) * 512)
                nc.tensor.matmul(att[:, s5],
                                 lhsT=vp_sb[:, kt, c0:c0 + V1],
                                 rhs=s_tiles[h][:, kt, s5],
                                 start=start, stop=stop)

        def evac(h, att, act):
            # one copy [65, QS]: rows 0..63 attended, row 64 normalizer
            fh = fpool.tile([V1, QS], bf16, tag="fh", name=f"fh{h}")
            if act:
                nc.scalar.activation(out=fh[:], in_=att[:], func=AF.Identity,
                                     bias=eps_col[:])
            else:
                nc.vector.tensor_scalar_add(out=fh[:], in0=att[:],
                                            scalar1=eps_col[:])
            r0 = (h % 2) * 64
            nc.sync.dma_start(out=flat[h // 2][r0:r0 + 64, :], in_=fh[0:64, :])
            # casting DMA (bf16 -> f32) must go through gpsimd
            nt = normsA if h >= 4 else normsB
            nc.gpsimd.dma_start(out=nt[h % 4:h % 4 + 1, :], in_=fh[64:65, :])

        # ---- phase A: dist matmul -> 3 exps per k-tile; d2 double-buffered
        # so ACT never waits on the PE.  The first two exp'd heads attend
        # in-phase (2 psB slots); chain squarings for the derived heads run
        # on DVE per 3-ktile group, hidden behind ACT.  The remaining exp'd
        # head and the derived heads attend after the loop.
        inphase = exp_heads[:2]
        postexp = exp_heads[2:]
        atts = {h: psB.tile([V1, QS], f32, tag="att", name=f"att{h}")
                for h in inphase}
        GK = 3
        dorder = [h for h in order if h in derived]
        for kt in range(KT):
            d2 = psA.tile([128, QS], f32, tag="ps")
            for qc in range(2):
                s5 = slice(qc * 512, (qc + 1) * 512)
                nc.tensor.matmul(d2[:, s5],
                                 lhsT=ka_sb[:, kt * 128:(kt + 1) * 128],
                                 rhs=qa_sb[:, s5], start=True, stop=True)
            for h in exp_heads:
                nc.scalar.activation(out=s_tiles[h][:, kt, :], in_=d2[:],
                                     func=AF.Exp, scale=float(cv[h]))
            for h in inphase:
                att_mms(atts[h], h, kt, start=(kt == 0), stop=(kt == KT - 1))
            if kt % GK == GK - 1 or kt == KT - 1:
                g0 = kt - kt % GK
                gs = slice(g0, kt + 1)
                gn = kt + 1 - g0
                for h in dorder:
                    src = s_tiles[derived[h]]
                    t = tmp.tile([128, gn, QS], bf16, tag="tmp",
                                 name=f"t{h}_{g0}")
                    nc.vector.tensor_mul(t[:], src[:, gs, :], src[:, gs, :])
                    nc.vector.tensor_mul(s_tiles[h][:, gs, :], t[:], t[:])
        for i, h in enumerate(inphase):
            evac(h, atts[h], act=(i == 0))

        # ---- poly heads: W[f, v] = sum_k psi[k, f] vaug[k, v]  (fills the
        # PE gap right after phase A)
        Wh = {}
        if poly:
            Wp = psA.tile([FT, PV], f32, tag="ps", name="Wp")
            for kt in range(KT):
                nc.tensor.matmul(Wp[:], lhsT=psi_sb[:, kt, :],
                                 rhs=vaug_sb[:, kt, :],
                                 start=(kt == 0), stop=(kt == KT - 1))
            W_sb = const.tile([FT, PV], bf16)
            nc.vector.tensor_copy(out=W_sb[:], in_=Wp[:])
            # per-head W slices shifted to partition 0 (DMA moves partitions)
            r0 = 0
            for i, h in enumerate(sorted(poly)):
                F = fdims[h]
                Wh[h] = const.tile([F, V1], bf16, name=f"W{h}")
                nc.sync.dma_start(out=Wh[h][:],
                                  in_=W_sb[r0:r0 + F, i * V1:(i + 1) * V1])
                r0 += F

        # ---- remaining attends.  The exp'd leftover + derived heads go
        # through psA (idle after phase A) so up to 4 attends are in
        # flight; poly attends rotate through psB ahead of the rb tiles.
        # poly attends first: they are cheap (2 matmuls each) and their
        # norms gate the first reciprocal group, so get them out of the way
        for h in sorted(poly):
            att = psB.tile([V1, QS], f32, tag="att", name=f"att{h}")
            for qc in range(2):
                s5 = slice(qc * 512, (qc + 1) * 512)
                nc.tensor.matmul(att[:, s5], lhsT=Wh[h][:],
                                 rhs=phi_sb[h][:, s5],
                                 start=True, stop=True)
            evac(h, att, act=(h == max(poly)))
        for i, h in enumerate(postexp + dorder):
            att = psA.tile([V1, QS], f32, tag="ps", name=f"att{h}")
            for kt in range(KT):
                att_mms(att, h, kt, start=(kt == 0), stop=(kt == KT - 1))
            evac(h, att, act=(i != 1))

        # ---- normalization: r = 1/(norm + 1e-5) via exp(-ln(x)), done in
        # two row-group passes so early heads don't wait for late ones.
        # r is broadcast across partitions by stride-0 DMA (no PSUM/PE),
        # then flat is scaled on DVE (bf16 2x).
        flatn = [const.tile([128, QS], bf16, name=f"flatn{j}")
                 for j in range(4)]
        for p, (nt, rf, rt) in enumerate(((normsA, rfA, r_hiA),
                                          (normsB, rfB, r_hiB))):
            # r = 1/(norm + eps): table-free DVE approx (~18 bits) instead of
            # ACT Ln/Exp, which would swap activation tables (1.3us each)
            nc.vector.reciprocal_approx_fast(out=rf[:], in_=nt[:])
            nc.vector.tensor_copy(out=rt[:], in_=rf[:])
            for j in ((2, 3) if p == 0 else (0, 1)):
                rb = (psB if p == 0 else psA).tile(
                    [128, QS], f32, tag="att" if p == 0 else "ps",
                    name=f"rb{j}")
                for qc in range(2):
                    s5 = slice(qc * 512, (qc + 1) * 512)
                    nc.tensor.matmul(rb[:, s5], lhsT=sel4_sb[:, j, :],
                                     rhs=rt[:, s5], start=True, stop=True)
                nc.vector.tensor_mul(flatn[j][:], flat[j][:], rb[:])

        # ---- out projection: outT[o, q] = sum_hv wt[hv, o] * flatn[hv, q]
        oq = [nc.sync, nc.scalar, nc.gpsimd, nc.sync]
        pools = [psA, psA, psB, psB]
        for ot in range(4):
            po = pools[ot].tile([128, QS], f32,
                                tag="ps" if ot < 2 else "att", name=f"po{ot}")
            for j in [2, 3, 0, 1]:
                for qc in range(2):
                    s5 = slice(qc * 512, (qc + 1) * 512)
                    nc.tensor.matmul(po[:, s5],
                                     lhsT=wt_sb[:, j, ot * 128:(ot + 1) * 128],
                                     rhs=flatn[j][:, s5],
                                     start=(j == 2), stop=(j == 1))
            ob = obuf.tile([128, QS], bf16, tag="ob", name=f"ob{ot}")
            if ot % 2 == 0:
                nc.scalar.copy(out=ob[:], in_=po[:])
            else:
                nc.vector.tensor_copy(out=ob[:], in_=po[:])
            oq[ot].dma_start(out=outT[ot * 128:(ot + 1) * 128, :], in_=ob[:])

    nc.compile()
    _cache[key] = nc
    return nc


def _hilo(x, bf16):
    hi = x.astype(bf16)
    lo = (x - hi.astype(np.float32)).astype(bf16)
    return hi, lo


def _prep_batch(kpos, vv, KT, cvf, poly, order, fdims, bf16):
    """Per-batch (key-side) tensors: ka, vp, vaug, psi."""
    Kp = KT * 128
    ncnt = kpos.shape[0]
    NS = len(order)
    k2 = (kpos * kpos).sum(-1)
    ka5 = np.zeros((5, Kp), np.float32)
    ka5[0:3, :ncnt] = kpos.T
    ka5[3, :ncnt] = k2
    ka5[4, :ncnt] = 1.0
    ka_hi, ka_lo = _hilo(ka5, bf16)
    ka = np.concatenate([ka_hi, ka_lo, ka_hi])   # [15, Kp]

    # score-head values (+ones), padded, [128, KT, NS*V1] fp16
    vs = np.zeros((Kp, NS, V1), np.float32)
    for i, h in enumerate(order):
        vs[:ncnt, i, :V] = vv[:, h, :]
    vs[:ncnt, :, V] = 1.0
    vp = vs.reshape(KT, 128, NS * V1).transpose(1, 0, 2).astype(bf16)

    # poly-head aug values + features
    ph = sorted(poly)
    va = np.zeros((Kp, len(ph), V1), np.float32)
    for i, h in enumerate(ph):
        va[:ncnt, i, :V] = vv[:, h, :]
    va[:ncnt, :, V] = 1.0
    vaug = va.reshape(KT, 128, len(ph) * V1).transpose(1, 0, 2).astype(bf16)
    FT = sum(fdims[h] for h in ph)
    psi = np.zeros((Kp, FT), np.float32)
    c0 = 0
    for h in ph:
        psi[:ncnt, c0:c0 + fdims[h]] = _features(kpos, cvf[h], poly[h])
        c0 += fdims[h]
    psi = psi.reshape(KT, 128, FT).transpose(1, 0, 2).astype(bf16)
    out = {"ka": np.ascontiguousarray(ka), "vp": np.ascontiguousarray(vp)}
    if ph:
        out["vaug"] = np.ascontiguousarray(vaug)
        out["psi"] = np.ascontiguousarray(psi)
    return out


def _prep_core(qp, cvf, poly, fdims, bf16):
    """Per-core (query-side) tensors: qa, phi{h}."""
    q2 = (qp * qp).sum(-1)
    one_q = np.ones(QS, np.float32)
    qa5 = np.stack([2 * qp[:, 0], 2 * qp[:, 1], 2 * qp[:, 2], -one_q, -q2]) \
        .astype(np.float32)
    qa_hi, qa_lo = _hilo(qa5, bf16)
    qa = np.concatenate([qa_hi, qa_hi, qa_lo])   # [15, QS]
    out = {"qa": np.ascontiguousarray(qa)}
    for h in sorted(poly):
        out[f"phi{h}"] = np.ascontiguousarray(
            _features(qp, cvf[h], poly[h]).T.astype(bf16))
    return out


def kernel(query_positions, key_positions, values, masked_elements,
           lengthscales, w_out, _want_trace=False):
    import ml_dtypes
    from concourse.bass_utils import run_bass_kernel_spmd

    bf16 = ml_dtypes.bfloat16
    qp = np.asarray(query_positions, np.float32)
    kp = np.asarray(key_positions, np.float32)
    vals = np.asarray(values, np.float32)
    mask = np.asarray(masked_elements).astype(bool)
    ls = np.asarray(lengthscales, np.float32)
    w = np.asarray(w_out, np.float32)

    cvf = (1.0 / (ls.astype(np.float64) ** 2)).astype(np.float32)
    poly, score, exp_heads, derived = _chain_plan(cvf)
    order = _order_score_heads(exp_heads, derived)
    fdims = {h: len(_monomials(d)) for h, d in poly.items()}

    keeps = [np.where(~mask[b])[0] for b in range(B)]
    KT = max(1, int(np.ceil(max(len(k) for k in keeps) / 128)))

    nc = _build(tuple(float(x) for x in cvf), KT, poly, score, exp_heads,
                derived, fdims)

    # shared (head-side) tensors
    wt = np.ascontiguousarray(w.T).reshape(4, 128, OUTD) \
        .transpose(1, 0, 2).astype(bf16)
    # sel4[:, j, :] broadcasts the two r rows of flat-group j (rows are
    # head%4 within the high/low norm group) across 128 partitions
    sel4 = np.zeros((4, 4, 128), np.float32)
    for j in range(4):
        h0, h1 = 2 * j, 2 * j + 1
        sel4[h0 % 4, j, :64] = 1.0
        sel4[h1 % 4, j, 64:] = 1.0
    shared = {"wt": np.ascontiguousarray(wt), "sel4": sel4.astype(bf16)}

    batch_maps = []
    for b in range(B):
        batch_maps.append(_prep_batch(kp[b][keeps[b]], vals[b][keeps[b]],
                                      KT, cvf, poly, order, fdims, bf16))
    in_maps = []
    for c in range(NCORES):
        b, hf = c // 2, c % 2
        m = dict(shared)
        m.update(batch_maps[b])
        m.update(_prep_core(qp[b, hf * QS:(hf + 1) * QS], cvf, poly, fdims,
                            bf16))
        in_maps.append(m)
    res = run_bass_kernel_spmd(nc, in_maps, core_ids=list(range(NCORES)),
                               trace=_want_trace)
    out = np.empty((B, LQ, OUTD), np.float32)
    for c in range(NCORES):
        b, hf = c // 2, c % 2
        out[b, hf * QS:(hf + 1) * QS, :] = \
            res.results[c]["outT"].astype(np.float32).T
    if _want_trace:
        return out, res
    return out

